# revision 11
# baseline (speedup 1.0000x reference)
"""INT8-dynamic-quant GEMM (per-token act quant, per-channel weight quant,
int8 GEMM, dequant) on 8 trn2 NeuronCores.

Math: the int8 values (|v| <= 127) are carried exactly in bf16; bf16 matmuls
with fp32 PSUM accumulation reproduce the int32 GEMM exactly (partial sums
stay far below 2^24). Rounding uses the +1.5*2^23 magic-number trick on the
DVE (exact IEEE fp32), which is round-to-nearest-even like jnp.round.

Sharding: column-parallel over all 8 cores. Each core holds the FULL x
[8192, 4096] and an N-slice of w [1376, 4096]; it produces out[:, n-slice].

Per-core data flow (no DRAM scratch for the quantized tensors):
  * w prep: 11 row-tiles (10x128 + 96) -> DVE quant -> one-shot SBUF->SBUF
    xbar transpose into the RESIDENT wqt [128, 32, 1376] bf16. Per-tile w
    scales also go striped -> flat DRAM -> swr row -> rank-1 PE matmul ->
    swb [128, 1376] broadcast (ready per-tile, early).
  * x streams: one 128-row tile per GEMM iteration i, quantized on DVE,
    one-shot transposed (sync queue) into a rotating xqt tile, consumed, and
    dropped. No scratch.
  * startup (the DVE prep chain is the serial resource): x and w tiles are
    interleaved on the DVE; the PE starts after just (x0, w0) with
    FINE-GRAINED units: for i < FINE_I, each (i, w-tile t) is its own
    128-col psum accumulation, emitted in availability order. These units
    write out scaled only by sx (sw not yet known for all tiles); a
    re-dequant pass later multiplies those rows by swb in place.
  * steady state (i >= FINE_I): 3 psum chunks (512/512/352) per i,
    chunk-major, evicted per chunk (ACT scale=sx, DVE multiply by swb).
"""

import contextlib

import numpy as np

import concourse.bass as bass
from concourse import bacc
import concourse.mybir as mybir
import concourse.tile as tile
from concourse import bass_utils

F32 = mybir.dt.float32
BF16 = mybir.dt.bfloat16
P = 128
MAGIC = 1.5 * 2**23  # fp32 round-to-nearest-even forcing constant
EPS = 1e-5

# Full problem
M, K, N = 8192, 4096, 11008
NG = 8  # column-parallel across all cores
NC = N // NG  # per-core n-slice: 1376
KT = K // P  # 32 k tiles
MT = M // P  # 64 m tiles
# w row-tiles: 1376 = 10*128 + 96 (xbar transpose needs multiples of 16 rows)
W_ROWS = [P] * 10 + [96]
WT = len(W_ROWS)
W_OFF = [sum(W_ROWS[:t]) for t in range(WT)]
CHUNKS = [(0, 512), (512, 512), (1024, 352)]  # (n0, width) psum chunks

FINE_I = 5  # m-tiles processed as fine-grained units during w prep
LOOK = 2  # x-prep lookahead in the steady loop
# DVE prep order during phase A: interleave x tiles among the w chain
PLAN = ["x0", "w0", "w1", "x1", "w2", "x2", "w3", "w4", "x3",
        "w5", "w6", "x4", "w7", "w8", "w9", "w10"]


def quant_body(tc, x_ap, w_ap, out_ap):
    nc = tc.nc

    est = contextlib.ExitStack()
    with est:
        dram = est.enter_context(tc.tile_pool(name="dram", bufs=1, space="DRAM"))
        stage = est.enter_context(tc.tile_pool(name="stage", bufs=2))
        qrow = est.enter_context(tc.tile_pool(name="qrow", bufs=1))
        wqtp = est.enter_context(tc.tile_pool(name="wqtp", bufs=1))
        xqtp = est.enter_context(tc.tile_pool(name="xqtp", bufs=FINE_I + 2))
        small = est.enter_context(tc.tile_pool(name="small", bufs=4))
        scl = est.enter_context(tc.tile_pool(name="scl", bufs=1))
        obf = est.enter_context(tc.tile_pool(name="obf", bufs=6))
        obc = est.enter_context(tc.tile_pool(name="obc", bufs=4))
        psum = est.enter_context(tc.tile_pool(name="psum", bufs=7, space="PSUM"))
        pssc = est.enter_context(tc.tile_pool(name="pssc", bufs=1, space="PSUM"))

        swsc = dram.tile([NC], F32, tag="swsc")
        raw_fine = dram.tile([FINE_I * P, NC], F32, tag="rawfine")
        wqt = wqtp.tile([P, KT, NC], BF16, tag="wqt")
        sx_striped = scl.tile([P, MT], F32, tag="sx")
        sw_striped = scl.tile([P, WT], F32, tag="sw")
        swb = scl.tile([P, NC], F32, tag="swb")
        swr = scl.tile([1, NC], F32, tag="swr")
        ones_row = scl.tile([1, P], F32, tag="ones")
        nc.vector.memset(ones_row, 1.0)

        def quant_rows(src_rows, scale_col, read_engine, pr=P):
            """[pr, K] fp32 rows -> bf16 int-valued q tile + per-row scale."""
            tin = stage.tile([P, K], F32, tag="stage")
            read_engine.dma_start(tin[:pr, :], src_rows)
            mx = small.tile([P, 1], F32, tag="mx")
            nc.vector.tensor_reduce(
                mx[:pr], tin[:pr, :], axis=mybir.AxisListType.X,
                op=mybir.AluOpType.max, apply_absolute_value=True,
            )
            nc.vector.tensor_scalar_max(mx[:pr], mx[:pr], EPS)
            nc.vector.tensor_scalar_mul(scale_col, mx[:pr], 1.0 / 127.0)
            return tin

        def quant_round(tin, scale_col, pr=P):
            inv = small.tile([P, 1], F32, tag="inv")
            nc.vector.reciprocal(inv[:pr], scale_col)
            nc.vector.tensor_scalar(
                tin[:pr, :], tin[:pr, :], inv[:pr], MAGIC,
                mybir.AluOpType.mult, mybir.AluOpType.add,
            )
            q = qrow.tile([P, K], BF16, tag="qrow")
            nc.vector.tensor_scalar(
                q[:pr, :], tin[:pr, :], MAGIC, None, mybir.AluOpType.subtract,
            )
            return q

        def emit_w_prep(t):
            n0, pr = W_OFF[t], W_ROWS[t]
            col = sw_striped[:pr, t : t + 1]
            tin = quant_rows(w_ap[n0 : n0 + pr, :], col, nc.scalar, pr)
            # sw side chain: striped col -> flat DRAM -> swr row -> swb bcast
            nc.gpsimd.dma_start(
                swsc[n0 : n0 + pr].rearrange("(o p) -> p o", p=pr), col
            )
            nc.scalar.dma_start(
                swr[0:1, n0 : n0 + pr], swsc[n0 : n0 + pr][None, :]
            )
            scps = pssc.tile([P, 512], F32, tag="scps")
            nc.tensor.matmul(
                scps[:, :pr], lhsT=ones_row, rhs=swr[0:1, n0 : n0 + pr],
                start=True, stop=True,
            )
            nc.scalar.copy(swb[:, n0 : n0 + pr], scps[:, :pr])
            q = quant_round(tin, col, pr)
            nc.sync.dma_start_transpose(wqt[:, :, n0 : n0 + pr], q[:pr, :])

        def emit_x_prep(i):
            col = sx_striped[:, i : i + 1]
            tin = quant_rows(x_ap[i * P : (i + 1) * P, :], col, nc.gpsimd)
            q = quant_round(tin, col)
            xq = xqtp.tile([P, KT, P], BF16, tag="xqt", name=f"xqt{i}")
            nc.sync.dma_start_transpose(xq, q)
            return xq

        def gemm(ps, xq, n0, wd):
            for k in range(KT):
                nc.tensor.matmul(
                    ps[:, :wd], lhsT=xq[:, k, :], rhs=wqt[:, k, n0 : n0 + wd],
                    start=(k == 0), stop=(k == KT - 1),
                )

        def emit_fine_unit(i, t):
            n0, wd = W_OFF[t], W_ROWS[t]
            ps = psum.tile([P, 512], F32, tag="ps")
            gemm(ps, xq_tiles[i], n0, wd)
            ob = obf.tile([P, P], F32, tag="obf")
            nc.scalar.activation(
                ob[:, :wd], ps[:, :wd], mybir.ActivationFunctionType.Copy,
                scale=sx_striped[:, i : i + 1],
            )
            nc.scalar.dma_start(
                raw_fine[i * P : (i + 1) * P, n0 : n0 + wd], ob[:, :wd]
            )

        def emit_coarse_unit(i, c):
            n0, wd = CHUNKS[c]
            ps = psum.tile([P, 512], F32, tag="ps")
            gemm(ps, xq_tiles[i], n0, wd)
            ob = obc.tile([P, 512], F32, tag="obc")
            nc.scalar.activation(
                ob[:, :wd], ps[:, :wd], mybir.ActivationFunctionType.Copy,
                scale=sx_striped[:, i : i + 1],
            )
            nc.vector.tensor_tensor(
                ob[:, :wd], ob[:, :wd], swb[:, n0 : n0 + wd],
                mybir.AluOpType.mult,
            )
            nc.scalar.dma_start(
                out_ap[i * P : (i + 1) * P, n0 : n0 + wd], ob[:, :wd]
            )

        def emit_redequant(b):
            """Multiply the raw (sx-scaled) fine rows b*128.. by swb."""
            rb = stage.tile([P, NC], F32, tag="stage")
            nc.scalar.dma_start(rb, raw_fine[b * P : (b + 1) * P, :])
            nc.vector.tensor_tensor(rb, rb, swb, mybir.AluOpType.mult)
            nc.scalar.dma_start(out_ap[b * P : (b + 1) * P, :], rb)

        # ---- phase A: interleaved prep + availability-ordered fine units ----
        pos = {item: p for p, item in enumerate(PLAN)}
        by_avail = {}
        for i in range(FINE_I):
            for t in range(WT):
                a = max(pos[f"x{i}"], pos[f"w{t}"])
                by_avail.setdefault(a, []).append((i, t))

        xq_tiles = {}
        for p, item in enumerate(PLAN):
            if item[0] == "x":
                xq_tiles[int(item[1:])] = emit_x_prep(int(item[1:]))
            else:
                emit_w_prep(int(item[1:]))
            for (i, t) in by_avail.get(p, []):
                emit_fine_unit(i, t)

        # ---- phase B: steady coarse loop ----
        redeq_at = {FINE_I + 6 + 2 * b: b for b in range(FINE_I)}
        for j in range(FINE_I, min(FINE_I + LOOK, MT)):
            xq_tiles[j] = emit_x_prep(j)
        for i in range(FINE_I, MT):
            if i + LOOK < MT:
                xq_tiles[i + LOOK] = emit_x_prep(i + LOOK)
            for c in range(len(CHUNKS)):
                emit_coarse_unit(i, c)
            xq_tiles.pop(i)
            if i in redeq_at:
                emit_redequant(redeq_at[i])


def build_nc():
    nc = bacc.Bacc("TRN2", target_bir_lowering=False, debug=False, num_devices=8)
    x_d = nc.dram_tensor("x", [M, K], F32, kind="ExternalInput")
    w_d = nc.dram_tensor("w", [NC, K], F32, kind="ExternalInput")
    out_d = nc.dram_tensor("out", [M, NC], F32, kind="ExternalOutput")
    with tile.TileContext(nc) as tc:
        quant_body(tc, x_d.ap(), w_d.ap(), out_d.ap())
    nc.compile()
    return nc


_NC_CACHE = {}


def get_nc():
    if "nc" not in _NC_CACHE:
        _NC_CACHE["nc"] = build_nc()
    return _NC_CACHE["nc"]


def shard_inputs(x, w):
    return [
        {"x": x, "w": np.ascontiguousarray(w[c * NC : (c + 1) * NC])}
        for c in range(8)
    ]


def gather(results):
    out = np.empty((M, N), dtype=np.float32)
    for c, r in enumerate(results):
        out[:, c * NC : (c + 1) * NC] = r["out"]
    return out


def kernel(**inputs):
    x = np.ascontiguousarray(np.asarray(inputs["x"], dtype=np.float32))
    w = np.ascontiguousarray(np.asarray(inputs["w"], dtype=np.float32))
    assert x.shape == (M, K) and w.shape == (N, K)
    nc = get_nc()
    res = bass_utils.run_bass_kernel_spmd(
        nc, shard_inputs(x, w), core_ids=list(range(8))
    )
    return gather(res.results)


# revision 15
# speedup vs baseline: 1.0860x; 1.0860x over previous
"""INT8-dynamic-quant GEMM (per-token act quant, per-channel weight quant,
int8 GEMM, dequant) on 8 trn2 NeuronCores.

Math: the int8 values (|v| <= 127) are carried exactly in bf16; bf16 matmuls
with fp32 PSUM accumulation reproduce the int32 GEMM exactly (partial sums
stay far below 2^24). Rounding uses the +1.5*2^23 magic-number trick on the
DVE (exact IEEE fp32), which is round-to-nearest-even like jnp.round.

Sharding: column-parallel over all 8 cores. Each core holds the FULL x
[8192, 4096] and an N-slice of w [1376, 4096]; it produces out[:, n-slice].

Per-core data flow (no DRAM scratch for the quantized tensors):
  * w prep: 11 row-tiles (10x128 + 96) -> DVE quant -> one-shot SBUF->SBUF
    xbar transpose into the RESIDENT wqt [128, 32, 1376] bf16. Per-tile w
    scales also go striped -> flat DRAM -> swr row -> rank-1 PE matmul ->
    swb [128, 1376] broadcast (ready per-tile, early).
  * x streams: one 128-row tile per GEMM iteration i, quantized on DVE,
    one-shot transposed (sync queue) into a rotating xqt tile, consumed, and
    dropped. No scratch.
  * startup (the DVE prep chain is the serial resource): x and w tiles are
    interleaved on the DVE; the PE starts after just (x0, w0) with
    FINE-GRAINED units: for i < FINE_I, each (i, w-tile t) is its own
    128-col psum accumulation, emitted in availability order. These units
    write out scaled only by sx (sw not yet known for all tiles); a
    re-dequant pass later multiplies those rows by swb in place.
  * steady state (i >= FINE_I): 3 psum chunks (512/512/352) per i,
    chunk-major, evicted per chunk (ACT scale=sx, DVE multiply by swb).
"""

import contextlib

import numpy as np

import concourse.bass as bass
from concourse import bacc
import concourse.mybir as mybir
import concourse.tile as tile
from concourse import bass_utils

F32 = mybir.dt.float32
BF16 = mybir.dt.bfloat16
P = 128
MAGIC = 1.5 * 2**23  # fp32 round-to-nearest-even forcing constant
EPS = 1e-5

# Full problem
M, K, N = 8192, 4096, 11008
NG = 8  # column-parallel across all cores
NC = N // NG  # per-core n-slice: 1376
KT = K // P  # 32 k tiles
MT = M // P  # 64 m tiles
# w row-tiles: 1376 = 10*128 + 96 (xbar transpose needs multiples of 16 rows)
W_ROWS = [P] * 10 + [96]
WT = len(W_ROWS)
W_OFF = [sum(W_ROWS[:t]) for t in range(WT)]
CHUNKS = [(0, 512), (512, 512), (1024, 352)]  # (n0, width) psum chunks

FINE_I = 5  # m-tiles processed as fine-grained units during w prep
LOOK = 2  # x-prep lookahead in the steady loop
# DVE prep order during phase A: interleave x tiles among the w chain
PLAN = ["x0", "w0", "w1", "x1", "w2", "x2", "w3", "w4", "x3",
        "w5", "w6", "x4", "w7", "w8", "w9", "w10"]


def quant_body(tc, x_ap, w_ap, out_ap):
    nc = tc.nc

    est = contextlib.ExitStack()
    with est:
        dram = est.enter_context(tc.tile_pool(name="dram", bufs=1, space="DRAM"))
        stage = est.enter_context(tc.tile_pool(name="stage", bufs=2))
        qrow = est.enter_context(tc.tile_pool(name="qrow", bufs=1))
        wqtp = est.enter_context(tc.tile_pool(name="wqtp", bufs=1))
        xqtp = est.enter_context(tc.tile_pool(name="xqtp", bufs=FINE_I + 2))
        small = est.enter_context(tc.tile_pool(name="small", bufs=4))
        scl = est.enter_context(tc.tile_pool(name="scl", bufs=1))
        obf = est.enter_context(tc.tile_pool(name="obf", bufs=6))
        obc = est.enter_context(tc.tile_pool(name="obc", bufs=2))
        psum = est.enter_context(tc.tile_pool(name="psum", bufs=7, space="PSUM"))
        pssc = est.enter_context(tc.tile_pool(name="pssc", bufs=1, space="PSUM"))

        swsc = dram.tile([NC], F32, tag="swsc")
        raw_fine = dram.tile([FINE_I * P, NC], F32, tag="rawfine")
        wqt = wqtp.tile([P, KT, NC], BF16, tag="wqt")
        sx_striped = scl.tile([P, MT], F32, tag="sx")
        sw_striped = scl.tile([P, WT], F32, tag="sw")
        swb = scl.tile([P, NC], F32, tag="swb")
        swr = scl.tile([1, NC], F32, tag="swr")
        ones_row = scl.tile([1, P], F32, tag="ones")
        nc.vector.memset(ones_row, 1.0)

        def quant_rows(src_rows, scale_col, read_engine, pr=P):
            """[pr, K] fp32 rows -> bf16 int-valued q tile + per-row scale."""
            tin = stage.tile([P, K], F32, tag="stage")
            read_engine.dma_start(tin[:pr, :], src_rows)
            mx = small.tile([P, 1], F32, tag="mx")
            nc.vector.tensor_reduce(
                mx[:pr], tin[:pr, :], axis=mybir.AxisListType.X,
                op=mybir.AluOpType.max, apply_absolute_value=True,
            )
            nc.vector.tensor_scalar_max(mx[:pr], mx[:pr], EPS)
            nc.vector.tensor_scalar_mul(scale_col, mx[:pr], 1.0 / 127.0)
            return tin

        def quant_round(tin, scale_col, pr=P):
            inv = small.tile([P, 1], F32, tag="inv")
            nc.vector.reciprocal(inv[:pr], scale_col)
            nc.vector.tensor_scalar(
                tin[:pr, :], tin[:pr, :], inv[:pr], MAGIC,
                mybir.AluOpType.mult, mybir.AluOpType.add,
            )
            q = qrow.tile([P, K], BF16, tag="qrow")
            nc.vector.tensor_scalar(
                q[:pr, :], tin[:pr, :], MAGIC, None, mybir.AluOpType.subtract,
            )
            return q

        def emit_w_prep(t):
            n0, pr = W_OFF[t], W_ROWS[t]
            col = sw_striped[:pr, t : t + 1]
            tin = quant_rows(w_ap[n0 : n0 + pr, :], col, nc.scalar, pr)
            # sw side chain: striped col -> flat DRAM -> swr row -> swb bcast
            nc.gpsimd.dma_start(
                swsc[n0 : n0 + pr].rearrange("(o p) -> p o", p=pr), col
            )
            nc.scalar.dma_start(
                swr[0:1, n0 : n0 + pr], swsc[n0 : n0 + pr][None, :]
            )
            scps = pssc.tile([P, 512], F32, tag="scps")
            nc.tensor.matmul(
                scps[:, :pr], lhsT=ones_row, rhs=swr[0:1, n0 : n0 + pr],
                start=True, stop=True,
            )
            nc.scalar.copy(swb[:, n0 : n0 + pr], scps[:, :pr])
            q = quant_round(tin, col, pr)
            nc.sync.dma_start_transpose(wqt[:, :, n0 : n0 + pr], q[:pr, :])

        def emit_x_prep(i):
            col = sx_striped[:, i : i + 1]
            tin = quant_rows(x_ap[i * P : (i + 1) * P, :], col, nc.gpsimd)
            q = quant_round(tin, col)
            xq = xqtp.tile([P, KT, P], BF16, tag="xqt", name=f"xqt{i}")
            nc.sync.dma_start_transpose(xq, q)
            return xq

        def gemm(ps, xq, n0, wd):
            for k in range(KT):
                nc.tensor.matmul(
                    ps[:, :wd], lhsT=xq[:, k, :], rhs=wqt[:, k, n0 : n0 + wd],
                    start=(k == 0), stop=(k == KT - 1),
                )

        def emit_fine_unit(i, t):
            n0, wd = W_OFF[t], W_ROWS[t]
            ps = psum.tile([P, 512], F32, tag="ps")
            gemm(ps, xq_tiles[i], n0, wd)
            ob = obf.tile([P, P], F32, tag="obf")
            nc.scalar.activation(
                ob[:, :wd], ps[:, :wd], mybir.ActivationFunctionType.Copy,
                scale=sx_striped[:, i : i + 1],
            )
            nc.scalar.dma_start(
                raw_fine[i * P : (i + 1) * P, n0 : n0 + wd], ob[:, :wd]
            )

        def emit_coarse_gemm(i):
            """k-major across the 3 chunks: consecutive matmuls share lhsT,
            keeping PE weight loads amortized/hidden."""
            pss = [
                psum.tile([P, 512], F32, tag="ps", name=f"ps{c}")
                for c in range(len(CHUNKS))
            ]
            xq = xq_tiles[i]
            for k in range(KT):
                for c, (n0, wd) in enumerate(CHUNKS):
                    nc.tensor.matmul(
                        pss[c][:, :wd],
                        lhsT=xq[:, k, :],
                        rhs=wqt[:, k, n0 : n0 + wd],
                        start=(k == 0),
                        stop=(k == KT - 1),
                    )
            return pss

        def emit_coarse_dequant(i, pss):
            ob = obc.tile([P, NC], F32, tag="obc")
            for c, (n0, wd) in enumerate(CHUNKS):
                nc.scalar.activation(
                    ob[:, n0 : n0 + wd], pss[c][:, :wd],
                    mybir.ActivationFunctionType.Copy,
                    scale=sx_striped[:, i : i + 1],
                )
            nc.vector.tensor_tensor(ob, ob, swb, mybir.AluOpType.mult)
            nc.scalar.dma_start(out_ap[i * P : (i + 1) * P, :], ob)

        def emit_redequant(b):
            """Multiply the raw (sx-scaled) fine rows b*128.. by swb."""
            rb = stage.tile([P, NC], F32, tag="stage")
            nc.scalar.dma_start(rb, raw_fine[b * P : (b + 1) * P, :])
            nc.vector.tensor_tensor(rb, rb, swb, mybir.AluOpType.mult)
            nc.scalar.dma_start(out_ap[b * P : (b + 1) * P, :], rb)

        # ---- phase A: interleaved prep + availability-ordered fine units ----
        pos = {item: p for p, item in enumerate(PLAN)}
        by_avail = {}
        for i in range(FINE_I):
            for t in range(WT):
                a = max(pos[f"x{i}"], pos[f"w{t}"])
                by_avail.setdefault(a, []).append((i, t))

        xq_tiles = {}
        for p, item in enumerate(PLAN):
            if item[0] == "x":
                xq_tiles[int(item[1:])] = emit_x_prep(int(item[1:]))
            else:
                emit_w_prep(int(item[1:]))
            for (i, t) in by_avail.get(p, []):
                emit_fine_unit(i, t)

        # ---- phase B: steady coarse loop (dequant lagged one iteration so
        # DVE prep always runs ahead of the dequant multiplies) ----
        redeq_at = {FINE_I + 6 + 2 * b: b for b in range(FINE_I)}
        for j in range(FINE_I, min(FINE_I + LOOK, MT)):
            xq_tiles[j] = emit_x_prep(j)
        pending = None  # (i, pss) awaiting dequant
        for i in range(FINE_I, MT):
            if i + LOOK < MT:
                xq_tiles[i + LOOK] = emit_x_prep(i + LOOK)
            pss = emit_coarse_gemm(i)
            if pending is not None:
                emit_coarse_dequant(*pending)
                xq_tiles.pop(pending[0])
            pending = (i, pss)
            if i in redeq_at:
                emit_redequant(redeq_at[i])
        emit_coarse_dequant(*pending)


def build_nc():
    nc = bacc.Bacc("TRN2", target_bir_lowering=False, debug=False, num_devices=8)
    x_d = nc.dram_tensor("x", [M, K], F32, kind="ExternalInput")
    w_d = nc.dram_tensor("w", [NC, K], F32, kind="ExternalInput")
    out_d = nc.dram_tensor("out", [M, NC], F32, kind="ExternalOutput")
    with tile.TileContext(nc) as tc:
        quant_body(tc, x_d.ap(), w_d.ap(), out_d.ap())
    nc.compile()
    return nc


_NC_CACHE = {}


def get_nc():
    if "nc" not in _NC_CACHE:
        _NC_CACHE["nc"] = build_nc()
    return _NC_CACHE["nc"]


def shard_inputs(x, w):
    return [
        {"x": x, "w": np.ascontiguousarray(w[c * NC : (c + 1) * NC])}
        for c in range(8)
    ]


def gather(results):
    out = np.empty((M, N), dtype=np.float32)
    for c, r in enumerate(results):
        out[:, c * NC : (c + 1) * NC] = r["out"]
    return out


def kernel(**inputs):
    x = np.ascontiguousarray(np.asarray(inputs["x"], dtype=np.float32))
    w = np.ascontiguousarray(np.asarray(inputs["w"], dtype=np.float32))
    assert x.shape == (M, K) and w.shape == (N, K)
    nc = get_nc()
    res = bass_utils.run_bass_kernel_spmd(
        nc, shard_inputs(x, w), core_ids=list(range(8))
    )
    return gather(res.results)
